# revision 43
# baseline (speedup 1.0000x reference)
"""YOLO DetectionLayer decode kernel for 8 Trainium2 NeuronCores.

Input  x [32, 255, 76, 76] fp32 -> output [32, 17328, 85] fp32.

Key layout fact: per image, out[(hw*3+box)*85 + attr] = f(x[box*85+attr, hw]),
i.e. the output is exactly the transpose of the [255, 5776] channel-major
input with per-channel activations (sigmoid / exp) and an affine box decode.

Per core (4 images): load [255,5776] channel-major in bf16 (minus the 12
xy/wh channels, whose output columns are produced separately from a small
fp32 side tensor), sigmoid in place, TensorE-transpose 128-col chunks into
PSUM, evacuate into a cell-major fp32 SBUF staging tile, then store
contiguous [cells, 255] fp32 rows.

Input conf/class channels are shipped fp8-e4m3 (host downcast): quarters
their load traffic at full DMA descriptor efficiency (2944B rows). The
sigmoid reads fp8 and writes bf16 staging tiles (fp8 OUTPUT storage of
probabilities would breach the error gate; fp8 INPUT error passes through
the sigmoid derivative: measured 1.4e-2 absolute, 4.4e-3 relative). Output must stay fp32 - bf16 rows
would be 510B descriptors, under the 512B full-bandwidth threshold, so a
bf16 store saves nothing. The error-critical exp(wh) path keeps exact
fp32 inputs via the xr side tensor; bf16 only touches the conf/class
sigmoid inputs (measured +8e-4 absolute, combined rel err ~2.4e-3 vs the
2e-2 gate).

Box coords: two accumulating bf16 matmuls per 128-cell chunk compute all
12 corner columns for all 4 images at once:  psP = rbS_chunk.T @ mwS +
rbE_chunk.T @ mwE, where rbS holds sigmoid(xy) rows + grid-offset rows,
rbE holds exp(wh) rows, and the constant mw [98,128] bakes in channel
selection, x1y1/x2y2 duplication, +-anchor/(2*608) scaling and the
grid-offset add.  The accumulated output overwrites the 12 box-coord
columns of each output group.

Sharding: pure data parallel, batch 32 -> 8 cores x 4 images.
"""
import sys

sys.path.insert(0, '/opt/trn_rl_repo')

import numpy as np
import ml_dtypes

NCORES = 8
BPC = 4          # batch per core
NCH = 255
HW = 5776        # 76*76
NATT = 85
IMG = 608.0
XYS = 1.05
GRID = 76.0
ANCHOR_WH = np.array([[10.0, 13.0], [16.0, 30.0], [33.0, 23.0]], np.float32)

# free-dim halves, aligned to 128-cell chunk boundaries (23 + 22.125 chunks)
HALVES = [(0, 2944), (2944, 2832)]
NCHUNK = 46      # ceil(5776/128); last chunk is 16 cells

_CACHE = {}


def _legalize_waits(nc, mybir):
    """walrus core_v3 rejects >1 wait on most instructions (2 on
    EventSemaphore). Tile's final drain carries one wait per live semaphore;
    split the excess onto preceding EventSemaphore carrier instructions."""
    n_new = 0
    for func in nc.m.functions:
        for block in func.blocks:
            out, changed = [], False
            for inst in block.instructions:
                si = inst.sync_info
                if si is not None:
                    waits = list(si.on_wait or [])
                    cap = 2 if isinstance(inst, mybir.InstEventSemaphore) else 1
                    if len(waits) > cap:
                        keep, extra = waits[:cap], waits[cap:]
                        for i in range(0, len(extra), 2):
                            es = mybir.InstEventSemaphore(
                                name=f"{inst.name}-ws{i}", ins=[], outs=[])
                            es.engine = inst.engine
                            es.sync_info = mybir.SyncInfo(
                                on_wait=list(extra[i:i + 2]), on_update=[])
                            out.append(es)
                            n_new += 1
                        inst.sync_info = mybir.SyncInfo(
                            on_wait=keep, on_update=list(si.on_update or []))
                        changed = True
                out.append(inst)
            if changed:
                block.instructions[:] = out
    return n_new


def make_consts():
    """Host-precomputed constant tensors (identical on every core).

    mw [98,128] bf16: the box-decode mixing matrix, two stacked blocks.
      Rows 0:50 (the rbS tile, K of the first matmul): 12*b + box*4 + attr
      for raw-channel sigmoid rows (attr 0:2 used), 48+ch for the
      grid-offset rows. Rows 50:98 (the rbE tile, K of the second matmul):
      12*b + box*4 + attr for exp rows (attr 2:4 used). The two matmuls
      accumulate into one PSUM tile (start/stop flags) - no partition-range
      gaps, nothing uninitialized is read.
      psP col layout (matches the evacuate src rearrange):
      j = 32*b + dup*6 + box*2 + ch.
    g [2,HW] bf16: normalized grid offsets (gx-0.025)/76, (gy-0.025)/76.
      psP col layout (matches the evacuate src rearrange):
      j = 32*b + dup*6 + box*2 + ch.
    """
    cell = np.arange(HW, dtype=np.float64)
    gx = (cell % 76 - 0.5 * (XYS - 1.0)) / GRID
    gy = (cell // 76 - 0.5 * (XYS - 1.0)) / GRID
    g = np.stack([gx, gy]).astype(ml_dtypes.bfloat16)

    mw = np.zeros((98, 128), np.float32)
    for b in range(BPC):
        for dup in range(2):
            for box in range(3):
                for ch in range(2):
                    j = 32 * b + dup * 6 + box * 2 + ch
                    mw[12 * b + box * 4 + ch, j] = XYS / GRID
                    sgn = -1.0 if dup == 0 else 1.0
                    mw[50 + 12 * b + box * 4 + 2 + ch, j] = (
                        sgn * ANCHOR_WH[box, ch] / (2.0 * IMG))
                    mw[48 + ch, j] = 1.0
    mw = mw.astype(ml_dtypes.bfloat16)
    idb = np.eye(128, dtype=np.float32).astype(ml_dtypes.bfloat16)
    return mw, g, idb


def _build(niter=1):
    import concourse.bass as bass
    import concourse.mybir as mybir
    from concourse.tile import TileContext
    from concourse import masks

    F32 = mybir.dt.float32
    BF16 = mybir.dt.bfloat16
    F8 = mybir.dt.float8e4
    AF = mybir.ActivationFunctionType

    nc = bass.Bass("TRN2")
    xb = nc.dram_tensor("xb", [BPC, NCH, 76, 76], F8, kind="ExternalInput")
    xr = nc.dram_tensor("xr", [BPC, 3, 4, HW], F32, kind="ExternalInput")
    mw = nc.dram_tensor("mw", [98, 128], BF16, kind="ExternalInput")
    g = nc.dram_tensor("g", [2, HW], BF16, kind="ExternalInput")
    idw = nc.dram_tensor("idw", [128, 128], BF16, kind="ExternalInput")
    out = nc.dram_tensor("out", [BPC, HW * 3, NATT], F32, kind="ExternalOutput")

    xf = xb[:].rearrange("b c h w -> b c (h w)")                 # [4,255,5776]
    out2 = out[:].rearrange("b r a -> b (r a)")                  # [4,1473840]

    with TileContext(nc) as tc:
        with tc.tile_pool(name="const", bufs=1) as cpool, \
             tc.tile_pool(name="rbp", bufs=1) as rbpool:
            ident = cpool.tile([128, 128], BF16)
            nc.scalar.dma_start(out=ident[:], in_=idw[:])
            mtS = cpool.tile([50, 128], BF16)
            mtE = cpool.tile([48, 128], BF16)
            nc.scalar.dma_start(out=mtS[:], in_=mw[0:50, :])
            nc.scalar.dma_start(out=mtE[:], in_=mw[50:98, :])

            for it in range(niter):
                # --------- box-coord raw loads (per half) -----
                # rr: raw xy/wh channels, 12 rows per image.
                # rb: sigmoid rows 0:48, exp rows 64:112, grid rows 112:114.
                # Loads ride the Pool/SWDGE ring so they don't clog the ACT
                # sequencer (whose HWDGE dispatch contends with SP loads).
                # per-half tags: both halves' rb tiles are live (read by
                # matmuls) for the whole image loop, so they must not share
                # a rotation slot - that creates an in-order PE queue cycle
                # (deadlock).
                rrs, rbSs, rbEs = [], [], []
                for hx, (h0, hw_) in enumerate(HALVES):
                    rr = rbpool.tile([48, 2944], F32, tag=f"rr{hx}")
                    rbS = rbpool.tile([50, 2944], BF16, tag=f"rbS{hx}")
                    rbE = rbpool.tile([48, 2944], BF16, tag=f"rbE{hx}")
                    nc.gpsimd.dma_start(out=rbS[48:50, :hw_],
                                        in_=g[:, h0:h0 + hw_])
                    # dst must stay a plain partition slice: a rearranged
                    # dst lets the AP optimizer merge partition+free dims,
                    # which HW descriptor generation mislowers. dma_start
                    # only checks total size, so nested DRAM srcs pair
                    # fine - one 48-row DMA covers all 4 images.
                    nc.sync.dma_start(out=rr[0:48, :hw_],
                                      in_=xr[:, :, :, h0:h0 + hw_])
                    rrs.append(rr)
                    rbSs.append(rbS)
                    rbEs.append(rbE)

                def emit_rb_acts(hx):
                    hw_ = HALVES[hx][1]
                    nc.scalar.activation(rbSs[hx][0:48, :hw_],
                                         rrs[hx][:, :hw_], AF.Sigmoid)
                    nc.scalar.activation(rbEs[hx][0:48, :hw_],
                                         rrs[hx][:, :hw_], AF.Exp)

                # rb-h0 activations up front; rb-h1 deferred until image 0's
                # h1 section so the first store chain isn't queued behind
                # them on the in-order ACT sequencer.
                emit_rb_acts(0)

                # chunk-group bounds per half (chunk 23 = first h1 chunk)
                HBOUNDS = [[0, 8, 16, 23], [23, 31, 39, 46]]

                # ---------------- main per-image pipeline ----------------
                with tc.tile_pool(name="t0r", bufs=4) as t0rpool, \
                     tc.tile_pool(name="t1r", bufs=4) as t1rpool, \
                     tc.tile_pool(name="t0", bufs=2) as t0pool, \
                     tc.tile_pool(name="t1", bufs=2) as t1pool, \
                     tc.tile_pool(name="og", bufs=6) as ogpool, \
                     tc.tile_pool(name="psX", bufs=3, space="PSUM") as psXpool, \
                     tc.tile_pool(name="psP", bufs=2, space="PSUM") as psPpool:
                    for b in range(BPC):
                        for hx, (h0, hw_) in enumerate(HALVES):
                            t0r = t0rpool.tile([128, 2944], F8, tag=f"t0r{hx}")
                            t1r = t1rpool.tile([127, 2944], F8, tag=f"t1r{hx}")
                            t0 = t0pool.tile([128, 2944], BF16, tag=f"t0{hx}")
                            t1 = t1pool.tile([127, 2944], BF16, tag=f"t1{hx}")
                            # rows 0:4 are skipped (their output columns
                            # come from psP); the other xy/wh rows (85:89,
                            # 42:46) load as fp8 junk inside one big DMA -
                            # their transposed output columns are
                            # overwritten from psP anyway. Fewer, bigger
                            # DMAs keep the dispatch path (SEQ+HWDGE) off
                            # the critical path in the fp8 era.
                            nc.sync.dma_start(out=t0r[4:128, :hw_],
                                              in_=xf[b, 4:128, h0:h0 + hw_])
                            nc.sync.dma_start(out=t1r[0:127, :hw_],
                                              in_=xf[b, 128:255, h0:h0 + hw_])
                            if b == 0 and hx == 1:
                                emit_rb_acts(1)

                            bounds = HBOUNDS[hx]
                            for og in range(len(bounds) - 1):
                                j0 = bounds[og]
                                j1 = bounds[og + 1]
                                # per-group sigmoid column slice: the group's
                                # transposes wait only on their own cells, not
                                # the whole half. Full-tile rows: 0:4 / 85:89
                                # / 42:46 are stale (channels not loaded), but
                                # their transposed output columns are
                                # overwritten from psP, so sigmoid(garbage)
                                # never reaches out.
                                sc0 = j0 * 128 - h0
                                sc1 = min(j1 * 128, HW) - h0
                                nc.scalar.activation(t0[:, sc0:sc1],
                                                     t0r[:, sc0:sc1],
                                                     AF.Sigmoid)
                                nc.scalar.activation(t1[:, sc0:sc1],
                                                     t1r[:, sc0:sc1],
                                                     AF.Sigmoid)
                                O = ogpool.tile([128, 2040], F32)
                                for g4 in range(j0, j1, 4):
                                    jj = list(range(g4, min(g4 + 4, j1)))
                                    n = len(jj)
                                    psX = psXpool.tile([128, 1024], BF16)
                                    psP = psPpool.tile([128, 512], F32)
                                    for k, j in enumerate(jj):
                                        c0 = j * 128
                                        w = min(128, HW - c0)
                                        ch0 = c0 - h0
                                        nc.tensor.transpose(
                                            psX[:w, k * 256:k * 256 + 128],
                                            t0[:, ch0:ch0 + w], ident[:, :])
                                        nc.tensor.transpose(
                                            psX[:w, k * 256 + 128:
                                                k * 256 + 255],
                                            t1[:, ch0:ch0 + w],
                                            ident[:127, :127])
                                        nc.tensor.matmul(
                                            psP[:w, k * 128:k * 128 + 128],
                                            rbSs[hx][:, ch0:ch0 + w],
                                            mtS[:, :], start=True, stop=False)
                                        nc.tensor.matmul(
                                            psP[:w, k * 128:k * 128 + 128],
                                            rbEs[hx][:, ch0:ch0 + w],
                                            mtE[:, :], start=False, stop=True)
                                    m = g4 - j0
                                    full = all(min(128, HW - j * 128) == 128
                                               for j in jj)
                                    if full:
                                        o3 = O[:, m * 255:(m + n) * 255].rearrange(
                                            "p (k a) -> p k a", a=255)
                                        sx = psX[:, :n * 256].rearrange(
                                            "p (k a) -> p k a", a=256)
                                        nc.vector.tensor_copy(
                                            o3, sx[:, :, 0:255])
                                        dst = O[:, m * 255:(m + n) * 255].rearrange(
                                            "p (k box r) -> p k box r", box=3, r=85
                                        )[:, :, :, 0:4].rearrange(
                                            "p k box (dup ch) -> p k box dup ch",
                                            dup=2)
                                        src = psP[:, :n * 128].rearrange(
                                            "p (k z) -> p k z", z=128
                                        )[:, :, 32 * b:32 * b + 12].rearrange(
                                            "p k (dup box ch) -> p k box dup ch",
                                            dup=2, box=3)
                                        nc.vector.tensor_copy(dst, src)
                                    else:
                                        for k, j in enumerate(jj):
                                            w = min(128, HW - j * 128)
                                            ok = O[:, (m + k) * 255:(m + k + 1) * 255]
                                            nc.vector.tensor_copy(
                                                ok[:w, 0:255],
                                                psX[:w, k * 256:k * 256 + 255])
                                            dst = ok[:w, :].rearrange(
                                                "p (box r) -> p box r", box=3, r=85
                                            )[:, :, 0:4].rearrange(
                                                "p box (dup ch) -> p box dup ch",
                                                dup=2)
                                            src = psP[:w, k * 128 + 32 * b:
                                                      k * 128 + 32 * b + 12].rearrange(
                                                "p (dup box ch) -> p box dup ch",
                                                dup=2, box=3)
                                            nc.vector.tensor_copy(dst, src)
                                # store this output group (ACT HWDGE ring, so
                                # the next loads on the SP ring aren't stuck
                                # behind stores in the same FIFO). The very
                                # last group stores per-g4 so the final DMA
                                # transfer (gating kernel end) is small.
                                last_group = (b == BPC - 1 and j1 == NCHUNK)
                                spans = ([(sp, min(sp + 4, j1))
                                          for sp in range(j0, j1, 4)]
                                         if last_group else [(j0, j1)])
                                for (sp0, sp1) in spans:
                                    sf = min(sp1, 45)  # full chunks only
                                    cell0 = sp0 * 128
                                    nfull = (sf - sp0) * 128
                                    m0 = (sp0 - j0) * 255
                                    dst = out2[b, cell0 * 255:
                                               (cell0 + nfull) * 255
                                               ].rearrange("(k p a) -> p k a",
                                                           p=128, a=255)
                                    nc.gpsimd.dma_start(
                                        out=dst,
                                        in_=O[:, m0:m0 + (sf - sp0) * 255
                                              ].rearrange("p (k a) -> p k a",
                                                          a=255))
                                    if sp1 == NCHUNK:  # 16-cell tail chunk
                                        dst2 = out2[b, 5760 * 255:5776 * 255
                                                    ].rearrange("(p a) -> p a",
                                                                a=255)
                                        nc.gpsimd.dma_start(
                                            out=dst2,
                                            in_=O[0:16, (45 - j0) * 255:
                                                  (46 - j0) * 255])

    _legalize_waits(nc, mybir)
    return nc


def _get_built(niter=1):
    if niter not in _CACHE:
        _CACHE[niter] = _build(niter)
    return _CACHE[niter]


def run_on_cores(x, niter=1):
    from concourse import bass_utils
    nc = _get_built(niter)
    mw, g, idb = make_consts()
    x8 = np.ascontiguousarray(np.asarray(x, np.float32).reshape(
        NCORES, BPC, NCH, 76, 76))
    xb8 = x8.astype(ml_dtypes.float8_e4m3)
    xr8 = np.ascontiguousarray(
        x8.reshape(NCORES, BPC, 3, NATT, HW)[:, :, :, 0:4, :])
    in_maps = [{"xb": xb8[i], "xr": xr8[i], "mw": mw, "g": g, "idw": idb}
               for i in range(NCORES)]
    res = bass_utils.run_bass_kernel_spmd(nc, in_maps,
                                          core_ids=list(range(NCORES)))
    outs = np.stack([res.results[i]["out"] for i in range(NCORES)])
    return outs.reshape(NCORES * BPC, HW * 3, NATT)


def kernel(x):
    return run_on_cores(x, niter=1)


# revision 46
# speedup vs baseline: 1.0094x; 1.0094x over previous
"""YOLO DetectionLayer decode kernel for 8 Trainium2 NeuronCores.

Input  x [32, 255, 76, 76] fp32 -> output [32, 17328, 85] fp32.

Key layout fact: per image, out[(hw*3+box)*85 + attr] = f(x[box*85+attr, hw]),
i.e. the output is exactly the transpose of the [255, 5776] channel-major
input with per-channel activations (sigmoid / exp) and an affine box decode.

Per core (4 images): load [255,5776] channel-major in bf16 (minus the 12
xy/wh channels, whose output columns are produced separately from a small
fp32 side tensor), sigmoid in place, TensorE-transpose 128-col chunks into
PSUM, evacuate into a cell-major fp32 SBUF staging tile, then store
contiguous [cells, 255] fp32 rows.

Input conf/class channels are shipped fp8-e4m3 (host downcast): quarters
their load traffic at full DMA descriptor efficiency (2944B rows). The
sigmoid reads fp8 and writes bf16 staging tiles (fp8 OUTPUT storage of
probabilities would breach the error gate; fp8 INPUT error passes through
the sigmoid derivative: measured 1.4e-2 absolute, 4.4e-3 relative). Output must stay fp32 - bf16 rows
would be 510B descriptors, under the 512B full-bandwidth threshold, so a
bf16 store saves nothing. The error-critical exp(wh) path keeps exact
fp32 inputs via the xr side tensor; bf16 only touches the conf/class
sigmoid inputs (measured +8e-4 absolute, combined rel err ~2.4e-3 vs the
2e-2 gate).

Box coords: two accumulating bf16 matmuls per 128-cell chunk compute all
12 corner columns for all 4 images at once:  psP = rbS_chunk.T @ mwS +
rbE_chunk.T @ mwE, where rbS holds sigmoid(xy) rows + grid-offset rows,
rbE holds exp(wh) rows, and the constant mw [98,128] bakes in channel
selection, x1y1/x2y2 duplication, +-anchor/(2*608) scaling and the
grid-offset add.  The accumulated output overwrites the 12 box-coord
columns of each output group.

Sharding: pure data parallel, batch 32 -> 8 cores x 4 images.
"""
import sys

sys.path.insert(0, '/opt/trn_rl_repo')

import numpy as np
import ml_dtypes

NCORES = 8
BPC = 4          # batch per core
NCH = 255
HW = 5776        # 76*76
NATT = 85
IMG = 608.0
XYS = 1.05
GRID = 76.0
ANCHOR_WH = np.array([[10.0, 13.0], [16.0, 30.0], [33.0, 23.0]], np.float32)

# free-dim halves, aligned to 128-cell chunk boundaries (23 + 22.125 chunks)
HALVES = [(0, 2944), (2944, 2832)]
NCHUNK = 46      # ceil(5776/128); last chunk is 16 cells

_CACHE = {}


def _legalize_waits(nc, mybir):
    """walrus core_v3 rejects >1 wait on most instructions (2 on
    EventSemaphore). Tile's final drain carries one wait per live semaphore;
    split the excess onto preceding EventSemaphore carrier instructions."""
    n_new = 0
    for func in nc.m.functions:
        for block in func.blocks:
            out, changed = [], False
            for inst in block.instructions:
                si = inst.sync_info
                if si is not None:
                    waits = list(si.on_wait or [])
                    cap = 2 if isinstance(inst, mybir.InstEventSemaphore) else 1
                    if len(waits) > cap:
                        keep, extra = waits[:cap], waits[cap:]
                        for i in range(0, len(extra), 2):
                            es = mybir.InstEventSemaphore(
                                name=f"{inst.name}-ws{i}", ins=[], outs=[])
                            es.engine = inst.engine
                            es.sync_info = mybir.SyncInfo(
                                on_wait=list(extra[i:i + 2]), on_update=[])
                            out.append(es)
                            n_new += 1
                        inst.sync_info = mybir.SyncInfo(
                            on_wait=keep, on_update=list(si.on_update or []))
                        changed = True
                out.append(inst)
            if changed:
                block.instructions[:] = out
    return n_new


def make_consts():
    """Host-precomputed constant tensors (identical on every core).

    mw [98,128] bf16: the box-decode mixing matrix, two stacked blocks.
      Rows 0:50 (the rbS tile, K of the first matmul): 12*b + box*4 + attr
      for raw-channel sigmoid rows (attr 0:2 used), 48+ch for the
      grid-offset rows. Rows 50:98 (the rbE tile, K of the second matmul):
      12*b + box*4 + attr for exp rows (attr 2:4 used). The two matmuls
      accumulate into one PSUM tile (start/stop flags) - no partition-range
      gaps, nothing uninitialized is read.
      psP col layout (matches the evacuate src rearrange):
      j = 32*b + dup*6 + box*2 + ch.
    g [2,HW] bf16: normalized grid offsets (gx-0.025)/76, (gy-0.025)/76.
      psP col layout (matches the evacuate src rearrange):
      j = 32*b + dup*6 + box*2 + ch.
    """
    cell = np.arange(HW, dtype=np.float64)
    gx = (cell % 76 - 0.5 * (XYS - 1.0)) / GRID
    gy = (cell // 76 - 0.5 * (XYS - 1.0)) / GRID
    g = np.stack([gx, gy]).astype(ml_dtypes.bfloat16)

    mw = np.zeros((98, 128), np.float32)
    for b in range(BPC):
        for dup in range(2):
            for box in range(3):
                for ch in range(2):
                    j = 32 * b + dup * 6 + box * 2 + ch
                    mw[12 * b + box * 4 + ch, j] = XYS / GRID
                    sgn = -1.0 if dup == 0 else 1.0
                    mw[50 + 12 * b + box * 4 + 2 + ch, j] = (
                        sgn * ANCHOR_WH[box, ch] / (2.0 * IMG))
                    mw[48 + ch, j] = 1.0
    mw = mw.astype(ml_dtypes.bfloat16)
    idb = np.eye(128, dtype=np.float32).astype(ml_dtypes.bfloat16)
    return mw, g, idb


def _build(niter=1):
    import concourse.bass as bass
    import concourse.mybir as mybir
    from concourse.tile import TileContext
    from concourse import masks

    F32 = mybir.dt.float32
    BF16 = mybir.dt.bfloat16
    F8 = mybir.dt.float8e4
    AF = mybir.ActivationFunctionType

    nc = bass.Bass("TRN2")
    xb = nc.dram_tensor("xb", [BPC, NCH, 76, 76], F8, kind="ExternalInput")
    xr = nc.dram_tensor("xr", [BPC, 3, 4, HW], F32, kind="ExternalInput")
    mw = nc.dram_tensor("mw", [98, 128], BF16, kind="ExternalInput")
    g = nc.dram_tensor("g", [2, HW], BF16, kind="ExternalInput")
    idw = nc.dram_tensor("idw", [128, 128], BF16, kind="ExternalInput")
    out = nc.dram_tensor("out", [BPC, HW * 3, NATT], F32, kind="ExternalOutput")

    xf = xb[:].rearrange("b c h w -> b c (h w)")                 # [4,255,5776]
    out2 = out[:].rearrange("b r a -> b (r a)")                  # [4,1473840]

    with TileContext(nc) as tc:
        with tc.tile_pool(name="const", bufs=1) as cpool, \
             tc.tile_pool(name="rbp", bufs=1) as rbpool:
            ident = cpool.tile([128, 128], BF16)
            nc.scalar.dma_start(out=ident[:], in_=idw[:])
            mtS = cpool.tile([50, 128], BF16)
            mtE = cpool.tile([48, 128], BF16)
            nc.scalar.dma_start(out=mtS[:], in_=mw[0:50, :])
            nc.scalar.dma_start(out=mtE[:], in_=mw[50:98, :])

            for it in range(niter):
                # --------- box-coord raw loads (per half) -----
                # rr: raw xy/wh channels, 12 rows per image.
                # rb: sigmoid rows 0:48, exp rows 64:112, grid rows 112:114.
                # Loads ride the Pool/SWDGE ring so they don't clog the ACT
                # sequencer (whose HWDGE dispatch contends with SP loads).
                # per-half tags: both halves' rb tiles are live (read by
                # matmuls) for the whole image loop, so they must not share
                # a rotation slot - that creates an in-order PE queue cycle
                # (deadlock).
                rrs, rbSs, rbEs = [], [], []
                for hx, (h0, hw_) in enumerate(HALVES):
                    rr = rbpool.tile([48, 2944], F32, tag=f"rr{hx}")
                    rbS = rbpool.tile([50, 2944], BF16, tag=f"rbS{hx}")
                    rbE = rbpool.tile([48, 2944], BF16, tag=f"rbE{hx}")
                    nc.gpsimd.dma_start(out=rbS[48:50, :hw_],
                                        in_=g[:, h0:h0 + hw_])
                    # dst must stay a plain partition slice: a rearranged
                    # dst lets the AP optimizer merge partition+free dims,
                    # which HW descriptor generation mislowers. dma_start
                    # only checks total size, so nested DRAM srcs pair
                    # fine - one 48-row DMA covers all 4 images.
                    nc.sync.dma_start(out=rr[0:48, :hw_],
                                      in_=xr[:, :, :, h0:h0 + hw_])
                    rrs.append(rr)
                    rbSs.append(rbS)
                    rbEs.append(rbE)

                def emit_rb_acts(hx):
                    hw_ = HALVES[hx][1]
                    nc.scalar.activation(rbSs[hx][0:48, :hw_],
                                         rrs[hx][:, :hw_], AF.Sigmoid)
                    nc.scalar.activation(rbEs[hx][0:48, :hw_],
                                         rrs[hx][:, :hw_], AF.Exp)

                # rb-h0 activations up front; rb-h1 deferred until image 0's
                # h1 section so the first store chain isn't queued behind
                # them on the in-order ACT sequencer.
                emit_rb_acts(0)

                # chunk-group bounds per half (chunk 23 = first h1 chunk)
                HBOUNDS = [[0, 8, 16, 23], [23, 31, 39, 46]]

                # ---------------- main per-image pipeline ----------------
                with tc.tile_pool(name="t0r", bufs=4) as t0rpool, \
                     tc.tile_pool(name="t1r", bufs=4) as t1rpool, \
                     tc.tile_pool(name="t0", bufs=2) as t0pool, \
                     tc.tile_pool(name="t1", bufs=2) as t1pool, \
                     tc.tile_pool(name="og", bufs=6) as ogpool, \
                     tc.tile_pool(name="psX", bufs=3, space="PSUM") as psXpool, \
                     tc.tile_pool(name="psP", bufs=2, space="PSUM") as psPpool:
                    for b in range(BPC):
                        for hx, (h0, hw_) in enumerate(HALVES):
                            t0r = t0rpool.tile([128, 2944], F8, tag=f"t0r{hx}")
                            t1r = t1rpool.tile([127, 2944], F8, tag=f"t1r{hx}")
                            t0 = t0pool.tile([128, 2944], BF16, tag=f"t0{hx}")
                            t1 = t1pool.tile([127, 2944], BF16, tag=f"t1{hx}")
                            # rows 0:4 are skipped (their output columns
                            # come from psP); the other xy/wh rows (85:89,
                            # 42:46) load as fp8 junk inside one big DMA -
                            # their transposed output columns are
                            # overwritten from psP anyway. Fewer, bigger
                            # DMAs keep the dispatch path (SEQ+HWDGE) off
                            # the critical path in the fp8 era.
                            nc.sync.dma_start(out=t0r[4:128, :hw_],
                                              in_=xf[b, 4:128, h0:h0 + hw_])
                            nc.sync.dma_start(out=t1r[0:127, :hw_],
                                              in_=xf[b, 128:255, h0:h0 + hw_])
                            if b == 0 and hx == 1:
                                emit_rb_acts(1)

                            bounds = HBOUNDS[hx]
                            for og in range(len(bounds) - 1):
                                j0 = bounds[og]
                                j1 = bounds[og + 1]
                                # per-group sigmoid column slice: the group's
                                # transposes wait only on their own cells, not
                                # the whole half. Full-tile rows: 0:4 / 85:89
                                # / 42:46 are stale (channels not loaded), but
                                # their transposed output columns are
                                # overwritten from psP, so sigmoid(garbage)
                                # never reaches out.
                                sc0 = j0 * 128 - h0
                                sc1 = min(j1 * 128, HW) - h0
                                nc.scalar.activation(t0[:, sc0:sc1],
                                                     t0r[:, sc0:sc1],
                                                     AF.Sigmoid)
                                nc.scalar.activation(t1[:, sc0:sc1],
                                                     t1r[:, sc0:sc1],
                                                     AF.Sigmoid)
                                O = ogpool.tile([128, 2040], F32)
                                for g4 in range(j0, j1, 4):
                                    jj = list(range(g4, min(g4 + 4, j1)))
                                    n = len(jj)
                                    psX = psXpool.tile([128, 1024], BF16)
                                    psP = psPpool.tile([128, 512], F32)
                                    for k, j in enumerate(jj):
                                        c0 = j * 128
                                        w = min(128, HW - c0)
                                        ch0 = c0 - h0
                                        nc.tensor.transpose(
                                            psX[:w, k * 256:k * 256 + 128],
                                            t0[:, ch0:ch0 + w], ident[:, :])
                                        nc.tensor.transpose(
                                            psX[:w, k * 256 + 128:
                                                k * 256 + 255],
                                            t1[:, ch0:ch0 + w],
                                            ident[:127, :127])
                                        nc.tensor.matmul(
                                            psP[:w, k * 128:k * 128 + 128],
                                            rbSs[hx][:, ch0:ch0 + w],
                                            mtS[:, :], start=True, stop=False)
                                        nc.tensor.matmul(
                                            psP[:w, k * 128:k * 128 + 128],
                                            rbEs[hx][:, ch0:ch0 + w],
                                            mtE[:, :], start=False, stop=True)
                                    m = g4 - j0
                                    full = all(min(128, HW - j * 128) == 128
                                               for j in jj)
                                    if full:
                                        o3 = O[:, m * 255:(m + n) * 255].rearrange(
                                            "p (k a) -> p k a", a=255)
                                        sx = psX[:, :n * 256].rearrange(
                                            "p (k a) -> p k a", a=256)
                                        # last image's copies ride ACT (idle
                                        # once sigmoids finish) so the DVE
                                        # stream drains before the store
                                        # backlog empties
                                        if b == BPC - 1:
                                            nc.scalar.copy(o3, sx[:, :, 0:255])
                                        else:
                                            nc.vector.tensor_copy(
                                                o3, sx[:, :, 0:255])
                                        dst = O[:, m * 255:(m + n) * 255].rearrange(
                                            "p (k box r) -> p k box r", box=3, r=85
                                        )[:, :, :, 0:4].rearrange(
                                            "p k box (dup ch) -> p k box dup ch",
                                            dup=2)
                                        src = psP[:, :n * 128].rearrange(
                                            "p (k z) -> p k z", z=128
                                        )[:, :, 32 * b:32 * b + 12].rearrange(
                                            "p k (dup box ch) -> p k box dup ch",
                                            dup=2, box=3)
                                        nc.vector.tensor_copy(dst, src)
                                    else:
                                        for k, j in enumerate(jj):
                                            w = min(128, HW - j * 128)
                                            ok = O[:, (m + k) * 255:(m + k + 1) * 255]
                                            nc.vector.tensor_copy(
                                                ok[:w, 0:255],
                                                psX[:w, k * 256:k * 256 + 255])
                                            dst = ok[:w, :].rearrange(
                                                "p (box r) -> p box r", box=3, r=85
                                            )[:, :, 0:4].rearrange(
                                                "p box (dup ch) -> p box dup ch",
                                                dup=2)
                                            src = psP[:w, k * 128 + 32 * b:
                                                      k * 128 + 32 * b + 12].rearrange(
                                                "p (dup box ch) -> p box dup ch",
                                                dup=2, box=3)
                                            nc.vector.tensor_copy(dst, src)
                                # store this output group (ACT HWDGE ring, so
                                # the next loads on the SP ring aren't stuck
                                # behind stores in the same FIFO). The very
                                # last group stores per-g4 so the final DMA
                                # transfer (gating kernel end) is small.
                                last_group = (b == BPC - 1 and j1 == NCHUNK)
                                spans = ([(sp, min(sp + 4, j1))
                                          for sp in range(j0, j1, 4)]
                                         if last_group else [(j0, j1)])
                                for (sp0, sp1) in spans:
                                    sf = min(sp1, 45)  # full chunks only
                                    cell0 = sp0 * 128
                                    nfull = (sf - sp0) * 128
                                    m0 = (sp0 - j0) * 255
                                    dst = out2[b, cell0 * 255:
                                               (cell0 + nfull) * 255
                                               ].rearrange("(k p a) -> p k a",
                                                           p=128, a=255)
                                    nc.gpsimd.dma_start(
                                        out=dst,
                                        in_=O[:, m0:m0 + (sf - sp0) * 255
                                              ].rearrange("p (k a) -> p k a",
                                                          a=255))
                                    if sp1 == NCHUNK:  # 16-cell tail chunk
                                        dst2 = out2[b, 5760 * 255:5776 * 255
                                                    ].rearrange("(p a) -> p a",
                                                                a=255)
                                        nc.gpsimd.dma_start(
                                            out=dst2,
                                            in_=O[0:16, (45 - j0) * 255:
                                                  (46 - j0) * 255])

    _legalize_waits(nc, mybir)
    return nc


def _get_built(niter=1):
    if niter not in _CACHE:
        _CACHE[niter] = _build(niter)
    return _CACHE[niter]


def run_on_cores(x, niter=1):
    from concourse import bass_utils
    nc = _get_built(niter)
    mw, g, idb = make_consts()
    x8 = np.ascontiguousarray(np.asarray(x, np.float32).reshape(
        NCORES, BPC, NCH, 76, 76))
    xb8 = x8.astype(ml_dtypes.float8_e4m3)
    xr8 = np.ascontiguousarray(
        x8.reshape(NCORES, BPC, 3, NATT, HW)[:, :, :, 0:4, :])
    in_maps = [{"xb": xb8[i], "xr": xr8[i], "mw": mw, "g": g, "idw": idb}
               for i in range(NCORES)]
    res = bass_utils.run_bass_kernel_spmd(nc, in_maps,
                                          core_ids=list(range(NCORES)))
    outs = np.stack([res.results[i]["out"] for i in range(NCORES)])
    return outs.reshape(NCORES * BPC, HW * 3, NATT)


def kernel(x):
    return run_on_cores(x, niter=1)
